# revision 41
# baseline (speedup 1.0000x reference)
"""Trainium2 Bass kernel for nn_Attention_27118423507150.

Multi-head attention (N=4, C=512, L=2048, 8 heads, d=64) on 8 NeuronCores.

Sharding: core c -> batch n = c//2, head-group hg = c%2 (4 of the 8 heads).
Each core computes its 4 heads end-to-end plus its partial output projection
w_out[:, hg_slice] @ heads -> [512, 2048]. Host sums the two partials per
batch and adds the bias (pure unshard, no device collective needed).

Per-core dataflow (bf16 compute, fp32 PSUM accumulation):
  q_all = wqT.T @ x        (SCALE folded into wqT on host)    [256, 2048]
  k_all = wkT.T @ x                                           [256, 2048]
  vT    = x.T @ wvT        (computed directly transposed)     [2048, 260]
          (stored per 128-row tile as 4 head-blocks of 65 cols: 64 v-cols
           + a ones column -> fused softmax denominator)
  per head h, query chunk I (512), key tile J (128):
    simT[J]  = k_h[:, J].T @ q_h[:, I]      (PSUM, K=64)
    expT[J]  = exp(simT[J])                 (ScalarE, no max-subtract:
                                             sim ~ N(0,1), exp <= ~e^5)
    pv[I]   += vta[J][:, h].T @ expT[J]     rows 0..63 = out^T, row 64 = denom
  denominators: 1/denom broadcast to 64 rows via a tiny block-ones matmul,
  normalize, then y_partial = woT.T @ out_normT on device.

build_nc(reps=K) replicates the compute body K times over the same SBUF
inputs (loads constant) so wall-clock deltas between K variants measure
pure on-device kernel time.
"""

import numpy as np
import ml_dtypes

import concourse.bass as bass
import concourse.mybir as mybir
from concourse import bacc
import concourse.tile as tile
from concourse.bass_utils import run_bass_kernel_spmd

BF16 = mybir.dt.bfloat16
F32 = mybir.dt.float32
AF = mybir.ActivationFunctionType

N, C, L = 4, 512, 2048
HEADS_PER_CORE = 4
D = 64
HD = HEADS_PER_CORE * D          # 256 head-dims per core
SCALE = D ** -0.5
N_CORES = 8

CI = 512                          # query chunk
NJ = L // 128                     # 16 key tiles
J_GROUP = 2                       # key tiles per exp batch (PSUM-bank bound)


def _emit_pv(nc, pv, vta, h, expt, js):
    for jj, J in enumerate(js):
        nc.tensor.matmul(
            pv[:, :],
            vta[J][:, h * (D + 1):(h + 1) * (D + 1)],
            expt[:, jj * CI:(jj + 1) * CI],
            start=(J == 0), stop=(J == NJ - 1))


def _body(nc, pp, wp, ps, xt, wq_sb, wk_sb, wv_sb, wo_sb, bones_sb, out_d,
          skip_dma=False, perhead=False, wo4_sb=None):
    """One full per-core computation over SBUF-resident x and weights."""
    # ---- q/k projections: q_all = wqT.T @ x ------------------------
    # m=0 (heads 0,1) projected up front; m=1 units are interleaved into
    # the attention stream (aux psum) so ScalarE starts exp'ing early.
    q_sb = [pp.tile([128, L], BF16, name=f"q{m}", tag=f"q{m}", bufs=2)
            for m in range(2)]
    k_sb = [pp.tile([128, L], BF16, name=f"k{m}", tag=f"k{m}", bufs=2)
            for m in range(2)]

    def _proj_unit(wsb, dst, m, nn, tag):
        acc = ps.tile([128, 512], F32, name="ps_proj", tag=tag, bufs=2)
        for kc in range(4):
            nc.tensor.matmul(
                acc[:, :],
                wsb[kc][:, m * 128:(m + 1) * 128],
                xt[kc][:, nn * 512:(nn + 1) * 512],
                start=(kc == 0), stop=(kc == 3))
        nc.vector.tensor_copy(
            dst[m][:, nn * 512:(nn + 1) * 512], acc[:, :])

    _proj_unit(wk_sb, k_sb, 0, 0, "sim")
    _proj_unit(wq_sb, q_sb, 0, 0, "sim")
    stream_sched = {}
    for g, u in zip((0, 2, 4), [(wk_sb, k_sb, 0, nn) for nn in (1, 2, 3)]):
        stream_sched[g] = u
    for g, u in zip((6, 10, 14), [(wq_sb, q_sb, 0, nn) for nn in (1, 2, 3)]):
        stream_sched[g] = u
    m1 = [(wsb, dst, 1, nn) for nn in range(4)
          for wsb, dst in ((wq_sb, q_sb), (wk_sb, k_sb))]
    for i, u in enumerate(m1):
        stream_sched[18 + 5 * i] = u

    # ---- vT projection with ones-augmented head blocks -------------
    # vta[lt] layout: [128, 4*65]; per head block: 64 v cols + 1 ones.
    # Units are emitted just-in-time inside the first attention groups.
    vta = []

    def _vta_unit(lt):
        acc = ps.tile([128, HD], F32, name="ps_v", tag="pv", bufs=2)
        for kc in range(4):
            nc.tensor.matmul(
                acc[:, :],
                xt[kc][:, lt * 128:(lt + 1) * 128],
                wv_sb[kc][:, :],
                start=(kc == 0), stop=(kc == 3))
        t = pp.tile([128, HEADS_PER_CORE * (D + 1)], BF16,
                    name=f"vta{lt}", tag=f"vta{lt}", bufs=2)
        t3 = t.rearrange("p (h c) -> p h c", c=D + 1)
        nc.gpsimd.memset(t3[:, :, D:D + 1], 1.0)
        nc.vector.tensor_copy(
            t3[:, :, 0:D], acc.rearrange("p (h c) -> p h c", c=D))
        vta.append(t)

    # ---- attention --------------------------------------------------
    # outu: unnormalized out^T (f32); den4[I]: denominators per query chunk
    if perhead:
        outuh = [pp.tile([D + 1, L], F32, name=f"outuh{h}", tag=f"outuh{h}",
                         bufs=2) for h in range(HEADS_PER_CORE)]
        onormh = [pp.tile([D, L], BF16, name=f"onormh{h}", tag=f"onormh{h}",
                          bufs=2) for h in range(HEADS_PER_CORE)]
    else:
        outu = [pp.tile([128, L], F32, name=f"outu{m}", tag=f"outu{m}",
                        bufs=2)
                for m in range(2)]
        if skip_dma:
            for m in range(2):
                nc.gpsimd.memset(outu[m][:, :], 1.0)
        onorm = [pp.tile([128, L], BF16, name=f"onorm{m}", tag=f"onorm{m}",
                         bufs=2)
                 for m in range(2)]
    den4 = [pp.tile([HEADS_PER_CORE, CI], F32, name=f"den4_{I}",
                    tag=f"den{I}", bufs=2) for I in range(L // CI)]

    def _norm_y(I):
        recipf = pp.tile([HEADS_PER_CORE, CI], F32, name="recipf",
                         tag=f"recipf{I}", bufs=2)
        nc.vector.reciprocal_approx_fast(recipf[:, :], den4[I][:, :])
        recip = pp.tile([HEADS_PER_CORE, CI], BF16, name="recip",
                        tag=f"recip{I}", bufs=2)
        nc.vector.tensor_copy(recip[:, :], recipf[:, :])
        sl = slice(I * CI, (I + 1) * CI)
        if perhead:
            for h in range(HEADS_PER_CORE):
                bc = ps.tile([D, CI], F32, name="bc", tag="aux", bufs=2)
                nc.tensor.matmul(
                    bc[:, :],
                    bones_sb[:, h * D:(h + 1) * D],
                    recip[:, :], start=True, stop=True)
                nc.vector.tensor_mul(onormh[h][:, sl],
                                     outuh[h][0:D, sl], bc[:, :])
        else:
            for m in range(2):
                bc = ps.tile([128, CI], F32, name="bc", tag="aux", bufs=2)
                nc.tensor.matmul(
                    bc[:, :],
                    bones_sb[:, m * 128:(m + 1) * 128],
                    recip[:, :], start=True, stop=True)
                nc.vector.tensor_mul(onorm[m][:, sl], outu[m][:, sl],
                                     bc[:, :])
        for m in range(4):
            y_ps = ps.tile([128, CI], F32, name="y_ps", tag="aux", bufs=2)
            if perhead:
                for h in range(HEADS_PER_CORE):
                    nc.tensor.matmul(
                        y_ps[:, :],
                        wo4_sb[h][:, m * 128:(m + 1) * 128],
                        onormh[h][:, I * CI:(I + 1) * CI],
                        start=(h == 0), stop=(h == 3))
            else:
                for kc in range(2):
                    nc.tensor.matmul(
                        y_ps[:, :],
                        wo_sb[kc][:, m * 128:(m + 1) * 128],
                        onorm[kc][:, I * CI:(I + 1) * CI],
                        start=(kc == 0), stop=(kc == 1))
            y_sb = wp.tile([128, CI], F32, name="y_sb", tag="y", bufs=3)
            nc.vector.tensor_copy(y_sb[:, :], y_ps[:, :])
            nc.gpsimd.dma_start(
                out_d[m * 128:(m + 1) * 128, I * CI:(I + 1) * CI],
                y_sb[:, :])

    j_groups = [list(range(g, min(g + J_GROUP, NJ)))
                for g in range(0, NJ, J_GROUP)]
    # software pipeline: pv matmuls for exp-group g are emitted while the
    # sims of group g+1 issue, so PE's in-order queue never blocks on ACT.
    # The pv drain (stage copy + DMA) rides with the last group's flush.
    def _flush(p):
        pv, h, expt, js, last = p
        _emit_pv(nc, pv, vta, h, expt, js)
        if last is not None:
            I = last
            if perhead:
                nc.vector.tensor_copy(
                    outuh[h][:, I * CI:(I + 1) * CI], pv[:, :])
                nc.sync.dma_start(
                    den4[I][h:h + 1, :],
                    outuh[h][D:D + 1, I * CI:(I + 1) * CI])
            else:
                stage = wp.tile([D + 1, CI], F32, name="stage", tag="stage",
                                bufs=3)
                nc.vector.tensor_copy(stage[:, :], pv[:, :])
                if not skip_dma:
                    nc.sync.dma_start(
                        outu[h // 2][(h % 2) * D:(h % 2) * D + D,
                                     I * CI:(I + 1) * CI],
                        stage[0:D, :])
                nc.sync.dma_start(
                    den4[I][h:h + 1, :],
                    stage[D:D + 1, :])

    pending = None
    gcount = 0
    n_groups = HEADS_PER_CORE * (L // CI) * len(j_groups)
    assert max(stream_sched) < 2 * (L // CI) * len(j_groups)
    for h in range(HEADS_PER_CORE):
        po = (h % 2) * D
        q_ap = q_sb[h // 2]
        k_ap = k_sb[h // 2]
        for I in range(L // CI):
            pv = ps.tile([D + 1, CI], F32, name="pv", tag="pv", bufs=2)
            for gi, js in enumerate(j_groups):
                while len(vta) < min(NJ, 2 * gcount + J_GROUP):
                    _vta_unit(len(vta))
                simt = ps.tile([128, J_GROUP * CI], F32, name="simt",
                               tag="sim", bufs=2)
                for jj, J in enumerate(js):
                    nc.tensor.matmul(
                        simt[:, jj * CI:(jj + 1) * CI],
                        k_ap[po:po + D, J * 128:(J + 1) * 128],
                        q_ap[po:po + D, I * CI:(I + 1) * CI],
                        start=True, stop=True)
                expt = wp.tile([128, J_GROUP * CI], BF16, name="expt",
                               tag="expt", bufs=3)
                w = len(js) * CI
                nc.scalar.activation(expt[:, :w], simt[:, :w], AF.Exp)
                if pending is not None:
                    _flush(pending)
                if gcount in stream_sched:
                    wsb, dst, m, nn = stream_sched[gcount]
                    _proj_unit(wsb, dst, m, nn, "aux")
                if h == 3 and gi == 2 and I > 0:
                    _norm_y(I - 1)
                last = I if gi == len(j_groups) - 1 else None
                pending = (pv, h, expt, js, last)
                gcount += 1
    if pending is not None:
        _flush(pending)
        pending = None

    if True:
        _norm_y(3)


def build_nc(reps=1, finalize=True, loop=0, skip_dma=False, hints=False,
             perhead=False):  # noqa: C901
    nc = bacc.Bacc(None)
    x_d = nc.declare_dram_parameter("x", [C, L], BF16, isOutput=False)
    wqkv_d = nc.declare_dram_parameter("wqkv", [C, 3 * HD], BF16,
                                       isOutput=False)
    wot_d = nc.declare_dram_parameter("wot", [HD, C], BF16, isOutput=False)
    bones_d = nc.declare_dram_parameter("bones", [HEADS_PER_CORE, HD], BF16,
                                        isOutput=False)
    out_d = nc.declare_dram_parameter("out", [C, L], F32, isOutput=True)

    with tile.TileContext(nc) as tc:
        with (
            tc.tile_pool(name="persist", bufs=1) as pp,
            tc.tile_pool(name="work", bufs=3) as wp,
            tc.tile_pool(name="psum", bufs=1, space="PSUM") as ps,
        ):
            # ---- loads -------------------------------------------------
            wq_sb, wk_sb, wv_sb, xt = [], [], [], []
            for kc in range(4):
                t = pp.tile([128, 3 * HD], BF16, name=f"wqkv{kc}")
                nc.sync.dma_start(t[:, :], wqkv_d[kc * 128:(kc + 1) * 128, :])
                wq_sb.append(t[:, 0:HD])
                wk_sb.append(t[:, HD:2 * HD])
                wv_sb.append(t[:, 2 * HD:3 * HD])
                xt.append(pp.tile([128, L], BF16, name=f"xt{kc}"))
            for half in range(2):
                for kc in range(4):
                    sl = slice(half * 1024, (half + 1) * 1024)
                    nc.gpsimd.dma_start(xt[kc][:, sl],
                                        x_d[kc * 128:(kc + 1) * 128, sl])
            wo_sb = []
            for kc in range(2):
                t = pp.tile([128, C], BF16, name=f"wo{kc}")
                nc.sync.dma_start(t[:, :], wot_d[kc * 128:(kc + 1) * 128, :])
                wo_sb.append(t)
            if perhead:
                wo4_sb = []
                for h in range(HEADS_PER_CORE):
                    t = pp.tile([D, C], BF16, name=f"wo4_{h}")
                    nc.sync.dma_start(t[:, :],
                                      wot_d[h * D:(h + 1) * D, :])
                    wo4_sb.append(t)
            else:
                wo4_sb = None
            bones_sb = pp.tile([HEADS_PER_CORE, HD], BF16, name="bones_sb")
            nc.sync.dma_start(bones_sb[:, :], bones_d[:, :])

            if loop:
                eng = ((mybir.EngineType.PE, mybir.EngineType.Activation,
                        mybir.EngineType.DVE, mybir.EngineType.SP,
                        mybir.EngineType.Pool) if hints else ())
                with tc.For_i(0, loop, 1, hint_engines=eng):
                    _body(nc, pp, wp, ps, xt, wq_sb, wk_sb, wv_sb, wo_sb,
                          bones_sb, out_d, skip_dma=skip_dma,
                          perhead=perhead, wo4_sb=wo4_sb)
            else:
                for _rep in range(reps):
                    _body(nc, pp, wp, ps, xt, wq_sb, wk_sb, wv_sb, wo_sb,
                          bones_sb, out_d, skip_dma=skip_dma,
                          perhead=perhead, wo4_sb=wo4_sb)
    if finalize:
        nc.finalize()
    return nc


def make_in_maps(x, w_qkv, w_out):
    bf = ml_dtypes.bfloat16
    bones = np.zeros((HEADS_PER_CORE, HD), np.float32)
    for h in range(HEADS_PER_CORE):
        bones[h, h * D:(h + 1) * D] = 1.0
    in_maps = []
    for c in range(N_CORES):
        n, hg = c // 2, c % 2
        sl = slice(hg * 256, (hg + 1) * 256)
        wq = (w_qkv[0:512][sl] * SCALE).T
        wk = w_qkv[512:1024][sl].T
        wv = w_qkv[1024:1536][sl].T
        wo = w_out[:, sl].T
        in_maps.append({
            "x": np.ascontiguousarray(x[n]).astype(bf),
            "wqkv": np.ascontiguousarray(
                np.concatenate([wq, wk, wv], axis=1)).astype(bf),
            "wot": np.ascontiguousarray(wo).astype(bf),
            "bones": bones.astype(bf),
        })
    return in_maps


_CACHE = {}


def run(x, w_qkv, w_out, b_out, reps=1, loop=0, skip_dma=False, hints=False):
    key = f"nc{reps}_{loop}_{skip_dma}_{hints}"
    if key not in _CACHE:
        _CACHE[key] = build_nc(reps, loop=loop, skip_dma=skip_dma,
                               hints=hints)
    in_maps = make_in_maps(np.asarray(x), np.asarray(w_qkv),
                           np.asarray(w_out))
    res = run_bass_kernel_spmd(_CACHE[key], in_maps,
                               core_ids=list(range(N_CORES)))
    parts = [np.asarray(res.results[c]["out"], np.float32)
             for c in range(N_CORES)]
    y = np.stack([parts[2 * n] + parts[2 * n + 1] for n in range(N)])
    y = y + np.asarray(b_out, np.float32)[None, :, None]
    return y.astype(np.float32), res


def kernel(x, w_qkv, w_out, b_out):
    y, _ = run(x, w_qkv, w_out, b_out)
    return y


# revision 55
# speedup vs baseline: 1.0719x; 1.0719x over previous
"""Trainium2 Bass kernel for nn_Attention_27118423507150.

Multi-head attention (N=4, C=512, L=2048, 8 heads, d=64) on 8 NeuronCores.

Sharding: core c -> batch n = c//2, head-group hg = c%2 (4 of the 8 heads).
Each core computes its 4 heads end-to-end plus its partial output projection
w_out[:, hg_slice] @ heads -> [512, 2048]. Host sums the two partials per
batch and adds the bias (pure unshard, no device collective needed).

Per-core dataflow (bf16 compute, fp32 PSUM accumulation):
  q_all = wqT.T @ x        (SCALE folded into wqT on host)    [256, 2048]
  k_all = wkT.T @ x                                           [256, 2048]
  vT    = x.T @ wvT        (computed directly transposed)     [2048, 260]
          (stored per 128-row tile as 4 head-blocks of 65 cols: 64 v-cols
           + a ones column -> fused softmax denominator)
  per head h, query chunk I (512), key tile J (128):
    simT[J]  = k_h[:, J].T @ q_h[:, I]      (PSUM, K=64)
    expT[J]  = exp(simT[J])                 (ScalarE, no max-subtract:
                                             sim ~ N(0,1), exp <= ~e^5)
    pv[I]   += vta[J][:, h].T @ expT[J]     rows 0..63 = out^T, row 64 = denom
  denominators: 1/denom broadcast to 64 rows via a tiny block-ones matmul,
  normalize, then y_partial = woT.T @ out_normT on device.

build_nc(reps=K) replicates the compute body K times over the same SBUF
inputs (loads constant) so wall-clock deltas between K variants measure
pure on-device kernel time.
"""

import numpy as np
import ml_dtypes

import concourse.bass as bass
import concourse.mybir as mybir
from concourse import bacc
import concourse.tile as tile
from concourse.bass_utils import run_bass_kernel_spmd

BF16 = mybir.dt.bfloat16
F32 = mybir.dt.float32
AF = mybir.ActivationFunctionType

N, C, L = 4, 512, 2048
HEADS_PER_CORE = 4
D = 64
HD = HEADS_PER_CORE * D          # 256 head-dims per core
SCALE = D ** -0.5
N_CORES = 8

CI = 512                          # query chunk
NJ = L // 128                     # 16 key tiles
J_GROUP = 2                       # key tiles per exp batch (PSUM-bank bound)


def _emit_pv(nc, pv, vta, h, expt, js, pvswap=False):
    for jj, J in enumerate(js):
        if pvswap:
            for sub in range(4):
                nc.tensor.matmul(
                    pv[:, sub * (D + 1):(sub + 1) * (D + 1)],
                    expt[:, jj * CI + sub * 128:jj * CI + (sub + 1) * 128],
                    vta[J][:, h * (D + 1):(h + 1) * (D + 1)],
                    start=(J == 0), stop=(J == NJ - 1))
        else:
            nc.tensor.matmul(
                pv[:, :],
                vta[J][:, h * (D + 1):(h + 1) * (D + 1)],
                expt[:, jj * CI:(jj + 1) * CI],
                start=(J == 0), stop=(J == NJ - 1))


def _body(nc, pp, wp, ps, xt, wq_sb, wk_sb, wv_sb, wo_sb, bones_sb, out_d,
          skip_dma=False, perhead=False, wo4_sb=None, pvswap=False,
          ident_sb=None):
    """One full per-core computation over SBUF-resident x and weights."""
    # ---- q/k projections: q_all = wqT.T @ x ------------------------
    # m=0 (heads 0,1) projected up front; m=1 units are interleaved into
    # the attention stream (aux psum) so ScalarE starts exp'ing early.
    q_sb = [pp.tile([128, L], BF16, name=f"q{m}", tag=f"q{m}", bufs=2)
            for m in range(2)]
    k_sb = [pp.tile([128, L], BF16, name=f"k{m}", tag=f"k{m}", bufs=2)
            for m in range(2)]

    def _proj_unit(wsb, dst, m, nn, tag):
        acc = ps.tile([128, 512], F32, name="ps_proj", tag=tag, bufs=2)
        for kc in range(4):
            nc.tensor.matmul(
                acc[:, :],
                wsb[kc][:, m * 128:(m + 1) * 128],
                xt[kc][:, nn * 512:(nn + 1) * 512],
                start=(kc == 0), stop=(kc == 3))
        nc.vector.tensor_copy(
            dst[m][:, nn * 512:(nn + 1) * 512], acc[:, :])

    _proj_unit(wk_sb, k_sb, 0, 0, "sim")
    _proj_unit(wq_sb, q_sb, 0, 0, "sim")
    stream_sched = {}
    for g, u in zip((0, 2, 4), [(wk_sb, k_sb, 0, nn) for nn in (1, 2, 3)]):
        stream_sched[g] = u
    for g, u in zip((6, 10, 14), [(wq_sb, q_sb, 0, nn) for nn in (1, 2, 3)]):
        stream_sched[g] = u
    m1 = [(wsb, dst, 1, nn) for nn in range(4)
          for wsb, dst in ((wq_sb, q_sb), (wk_sb, k_sb))]
    for i, u in enumerate(m1):
        stream_sched[18 + 5 * i] = u

    # ---- vT projection with ones-augmented head blocks -------------
    # vta[lt] layout: [128, 4*65]; per head block: 64 v cols + 1 ones.
    # Units are emitted just-in-time inside the first attention groups.
    vta = []

    def _vta_unit(lt):
        acc = ps.tile([128, HD], F32, name="ps_v", tag="pv", bufs=2)
        for kc in range(4):
            nc.tensor.matmul(
                acc[:, :],
                xt[kc][:, lt * 128:(lt + 1) * 128],
                wv_sb[kc][:, :],
                start=(kc == 0), stop=(kc == 3))
        t = pp.tile([128, HEADS_PER_CORE * (D + 1)], BF16,
                    name=f"vta{lt}", tag=f"vta{lt}", bufs=2)
        t3 = t.rearrange("p (h c) -> p h c", c=D + 1)
        nc.gpsimd.memset(t3[:, :, D:D + 1], 1.0)
        nc.vector.tensor_copy(
            t3[:, :, 0:D], acc.rearrange("p (h c) -> p h c", c=D))
        vta.append(t)

    # ---- attention --------------------------------------------------
    # outu: unnormalized out^T (f32); den4[I]: denominators per query chunk
    if perhead:
        outuh = [pp.tile([D + 1, L], F32, name=f"outuh{h}", tag=f"outuh{h}",
                         bufs=2) for h in range(HEADS_PER_CORE)]
        onormh = [pp.tile([D, L], BF16, name=f"onormh{h}", tag=f"onormh{h}",
                          bufs=2) for h in range(HEADS_PER_CORE)]
    elif pvswap:
        onorm = [pp.tile([128, L], BF16, name=f"onorm{m}", tag=f"onorm{m}",
                         bufs=2)
                 for m in range(2)]
    else:
        outu = [pp.tile([128, L], F32, name=f"outu{m}", tag=f"outu{m}",
                        bufs=2)
                for m in range(2)]
        if skip_dma:
            for m in range(2):
                nc.gpsimd.memset(outu[m][:, :], 1.0)
        onorm = [pp.tile([128, L], BF16, name=f"onorm{m}", tag=f"onorm{m}",
                         bufs=2)
                 for m in range(2)]
    den4 = ([pp.tile([HEADS_PER_CORE, CI], F32, name=f"den4_{I}",
                     tag=f"den{I}", bufs=2) for I in range(L // CI)]
            if not pvswap else None)

    def _norm_y(I):
        if pvswap:
            _y_only(I, 0, CI)
            return
        recipf = pp.tile([HEADS_PER_CORE, CI], F32, name="recipf",
                         tag=f"recipf{I}", bufs=2)
        nc.vector.reciprocal_approx_fast(recipf[:, :], den4[I][:, :])
        recip = pp.tile([HEADS_PER_CORE, CI], BF16, name="recip",
                        tag=f"recip{I}", bufs=2)
        nc.vector.tensor_copy(recip[:, :], recipf[:, :])
        sl = slice(I * CI, (I + 1) * CI)
        if perhead:
            for h in range(HEADS_PER_CORE):
                bc = ps.tile([D, CI], F32, name="bc", tag="aux", bufs=2)
                nc.tensor.matmul(
                    bc[:, :],
                    bones_sb[:, h * D:(h + 1) * D],
                    recip[:, :], start=True, stop=True)
                nc.vector.tensor_mul(onormh[h][:, sl],
                                     outuh[h][0:D, sl], bc[:, :])
        else:
            for m in range(2):
                bc = ps.tile([128, CI], F32, name="bc", tag="aux", bufs=2)
                nc.tensor.matmul(
                    bc[:, :],
                    bones_sb[:, m * 128:(m + 1) * 128],
                    recip[:, :], start=True, stop=True)
                nc.vector.tensor_mul(onorm[m][:, sl], outu[m][:, sl],
                                     bc[:, :])
        for m in range(4):
            y_ps = ps.tile([128, CI], F32, name="y_ps", tag="aux", bufs=2)
            if perhead:
                for h in range(HEADS_PER_CORE):
                    nc.tensor.matmul(
                        y_ps[:, :],
                        wo4_sb[h][:, m * 128:(m + 1) * 128],
                        onormh[h][:, I * CI:(I + 1) * CI],
                        start=(h == 0), stop=(h == 3))
            else:
                for kc in range(2):
                    nc.tensor.matmul(
                        y_ps[:, :],
                        wo_sb[kc][:, m * 128:(m + 1) * 128],
                        onorm[kc][:, I * CI:(I + 1) * CI],
                        start=(kc == 0), stop=(kc == 1))
            y_sb = wp.tile([128, CI], F32, name="y_sb", tag="y", bufs=3)
            nc.vector.tensor_copy(y_sb[:, :], y_ps[:, :])
            nc.gpsimd.dma_start(
                out_d[m * 128:(m + 1) * 128, I * CI:(I + 1) * CI],
                y_sb[:, :])

    def _y_only(I, off, w):
        sl = slice(I * CI + off, I * CI + off + w)
        for m in range(4):
            y_ps = ps.tile([128, CI], F32, name="y_ps", tag="aux", bufs=2)
            for kc in range(2):
                nc.tensor.matmul(
                    y_ps[:, 0:w],
                    wo_sb[kc][:, m * 128:(m + 1) * 128],
                    onorm[kc][:, sl],
                    start=(kc == 0), stop=(kc == 1))
            y_sb = wp.tile([128, CI], F32, name="y_sb", tag="y", bufs=3)
            nc.vector.tensor_copy(y_sb[:, 0:w], y_ps[:, 0:w])
            nc.gpsimd.dma_start(
                out_d[m * 128:(m + 1) * 128, I * CI + off:I * CI + off + w],
                y_sb[:, 0:w])

    j_groups = [list(range(g, min(g + J_GROUP, NJ)))
                for g in range(0, NJ, J_GROUP)]
    # software pipeline: pv matmuls for exp-group g are emitted while the
    # sims of group g+1 issue, so PE's in-order queue never blocks on ACT.
    # The pv drain (stage copy + DMA) rides with the last group's flush.
    nrm = ([[pp.tile([128, 128], BF16, name=f"nrm{hp}_{lt}",
                     tag=f"nrm{hp}_{lt}", bufs=2) for lt in range(NJ)]
            for hp in range(2)] if pvswap else None)

    def _flush(p):
        pv, h, expt, js, last = p
        _emit_pv(nc, pv, vta, h, expt, js, pvswap=pvswap)
        if pvswap:
            if last is not None:
                I = last
                for sub in range(4):
                    lt = I * 4 + sub
                    c0 = sub * (D + 1)
                    rcp = wp.tile([128, 1], mybir.dt.float32, name="rcp",
                                  tag="rcp", bufs=4)
                    nc.vector.reciprocal(rcp[:, :], pv[:, c0 + D:c0 + D + 1])
                    nc.vector.tensor_scalar_mul(
                        nrm[h // 2][lt][:, (h % 2) * D:(h % 2) * D + D],
                        pv[:, c0:c0 + D], rcp[:, :])
                    if h % 2 == 1:
                        tp = ps.tile([128, 128], BF16, name="tp", tag="aux",
                                     bufs=2)
                        nc.tensor.transpose(tp[:, :], nrm[h // 2][lt][:, :],
                                            ident_sb[:, :])
                        nc.vector.tensor_copy(
                            onorm[h // 2][:, lt * 128:(lt + 1) * 128],
                            tp[:, :])
            return
        if last is not None:
            I = last
            if perhead:
                nc.vector.tensor_copy(
                    outuh[h][:, I * CI:(I + 1) * CI], pv[:, :])
                nc.sync.dma_start(
                    den4[I][h:h + 1, :],
                    outuh[h][D:D + 1, I * CI:(I + 1) * CI])
            else:
                stage = wp.tile([D + 1, CI], F32, name="stage", tag="stage",
                                bufs=3)
                nc.vector.tensor_copy(stage[:, :], pv[:, :])
                if not skip_dma:
                    nc.sync.dma_start(
                        outu[h // 2][(h % 2) * D:(h % 2) * D + D,
                                     I * CI:(I + 1) * CI],
                        stage[0:D, :])
                nc.sync.dma_start(
                    den4[I][h:h + 1, :],
                    stage[D:D + 1, :])

    pending = None
    pending2 = []
    gcount = 0
    n_groups = HEADS_PER_CORE * (L // CI) * len(j_groups)
    assert max(stream_sched) < 2 * (L // CI) * len(j_groups)
    for h in range(HEADS_PER_CORE):
        po = (h % 2) * D
        q_ap = q_sb[h // 2]
        k_ap = k_sb[h // 2]
        for I in range(L // CI):
            pv = ps.tile([128, 4 * (D + 1)] if pvswap else [D + 1, CI],
                         F32, name="pv", tag="pv", bufs=2)
            for gi, js in enumerate(j_groups):
                while len(vta) < min(NJ, 2 * gcount + J_GROUP):
                    _vta_unit(len(vta))
                simt = ps.tile([128, J_GROUP * CI], F32, name="simt",
                               tag="sim", bufs=2)
                for jj, J in enumerate(js):
                    nc.tensor.matmul(
                        simt[:, jj * CI:(jj + 1) * CI],
                        k_ap[po:po + D, J * 128:(J + 1) * 128],
                        q_ap[po:po + D, I * CI:(I + 1) * CI],
                        start=True, stop=True)
                expt = wp.tile([128, J_GROUP * CI], BF16, name="expt",
                               tag="expt", bufs=7)
                w = len(js) * CI
                nc.scalar.activation(expt[:, :w], simt[:, :w], AF.Exp)
                if len(pending2) == 6:
                    _flush(pending2.pop(0))
                if gcount in stream_sched:
                    wsb, dst, m, nn = stream_sched[gcount]
                    _proj_unit(wsb, dst, m, nn, "aux")
                if h == 3 and gi == 2 and I > 0:
                    _norm_y(I - 1)
                last = I if gi == len(j_groups) - 1 else None
                pending2.append((pv, h, expt, js, last))
                gcount += 1
    for p in pending2:
        _flush(p)
    pending2 = []

    if True:
        _norm_y(3)


def build_nc(reps=1, finalize=True, loop=0, skip_dma=False, hints=False,
             perhead=False, pvswap=False):  # noqa: C901
    nc = bacc.Bacc(None)
    x_d = nc.declare_dram_parameter("x", [C, L], BF16, isOutput=False)
    wqkv_d = nc.declare_dram_parameter("wqkv", [C, 3 * HD], BF16,
                                       isOutput=False)
    wot_d = nc.declare_dram_parameter("wot", [HD, C], BF16, isOutput=False)
    bones_d = nc.declare_dram_parameter("bones", [HEADS_PER_CORE, HD], BF16,
                                        isOutput=False)
    ident_d = (nc.declare_dram_parameter("ident", [128, 128], BF16,
                                         isOutput=False) if pvswap else None)
    out_d = nc.declare_dram_parameter("out", [C, L], F32, isOutput=True)

    with tile.TileContext(nc) as tc:
        with (
            tc.tile_pool(name="persist", bufs=1) as pp,
            tc.tile_pool(name="work", bufs=3) as wp,
            tc.tile_pool(name="psum", bufs=1, space="PSUM") as ps,
        ):
            # ---- loads -------------------------------------------------
            wq_sb, wk_sb, wv_sb, xt = [], [], [], []
            for kc in range(4):
                t = pp.tile([128, 3 * HD], BF16, name=f"wqkv{kc}")
                nc.sync.dma_start(t[:, :], wqkv_d[kc * 128:(kc + 1) * 128, :])
                wq_sb.append(t[:, 0:HD])
                wk_sb.append(t[:, HD:2 * HD])
                wv_sb.append(t[:, 2 * HD:3 * HD])
                xt.append(pp.tile([128, L], BF16, name=f"xt{kc}"))
            for half in range(2):
                for kc in range(4):
                    sl = slice(half * 1024, (half + 1) * 1024)
                    nc.gpsimd.dma_start(xt[kc][:, sl],
                                        x_d[kc * 128:(kc + 1) * 128, sl])
            wo_sb = []
            for kc in range(2):
                t = pp.tile([128, C], BF16, name=f"wo{kc}")
                nc.sync.dma_start(t[:, :], wot_d[kc * 128:(kc + 1) * 128, :])
                wo_sb.append(t)
            if perhead:
                wo4_sb = []
                for h in range(HEADS_PER_CORE):
                    t = pp.tile([D, C], BF16, name=f"wo4_{h}")
                    nc.sync.dma_start(t[:, :],
                                      wot_d[h * D:(h + 1) * D, :])
                    wo4_sb.append(t)
            else:
                wo4_sb = None
            bones_sb = pp.tile([HEADS_PER_CORE, HD], BF16, name="bones_sb")
            nc.sync.dma_start(bones_sb[:, :], bones_d[:, :])
            if pvswap:
                ident_sb = pp.tile([128, 128], BF16, name="ident_sb")
                nc.sync.dma_start(ident_sb[:, :], ident_d[:, :])
            else:
                ident_sb = None

            if loop:
                eng = ((mybir.EngineType.PE, mybir.EngineType.Activation,
                        mybir.EngineType.DVE, mybir.EngineType.SP,
                        mybir.EngineType.Pool) if hints else ())
                with tc.For_i(0, loop, 1, hint_engines=eng):
                    _body(nc, pp, wp, ps, xt, wq_sb, wk_sb, wv_sb, wo_sb,
                          bones_sb, out_d, skip_dma=skip_dma,
                          perhead=perhead, wo4_sb=wo4_sb, pvswap=pvswap,
                          ident_sb=ident_sb)
            else:
                for _rep in range(reps):
                    _body(nc, pp, wp, ps, xt, wq_sb, wk_sb, wv_sb, wo_sb,
                          bones_sb, out_d, skip_dma=skip_dma,
                          perhead=perhead, wo4_sb=wo4_sb, pvswap=pvswap,
                          ident_sb=ident_sb)
    if finalize:
        nc.finalize()
    return nc


def make_in_maps(x, w_qkv, w_out, pvswap=False):
    bf = ml_dtypes.bfloat16
    bones = np.zeros((HEADS_PER_CORE, HD), np.float32)
    for h in range(HEADS_PER_CORE):
        bones[h, h * D:(h + 1) * D] = 1.0
    in_maps = []
    for c in range(N_CORES):
        n, hg = c // 2, c % 2
        sl = slice(hg * 256, (hg + 1) * 256)
        wq = (w_qkv[0:512][sl] * SCALE).T
        wk = w_qkv[512:1024][sl].T
        wv = w_qkv[1024:1536][sl].T
        wo = w_out[:, sl].T
        in_maps.append({
            "x": np.ascontiguousarray(x[n]).astype(bf),
            "wqkv": np.ascontiguousarray(
                np.concatenate([wq, wk, wv], axis=1)).astype(bf),
            "wot": np.ascontiguousarray(wo).astype(bf),
            "bones": bones.astype(bf),
        })
        if pvswap:
            in_maps[-1]["ident"] = np.eye(128, dtype=bf)
    return in_maps


_CACHE = {}


def run(x, w_qkv, w_out, b_out, reps=1, loop=0, skip_dma=False, hints=False,
        pvswap=False):
    key = f"nc{reps}_{loop}_{skip_dma}_{hints}_{pvswap}"
    if key not in _CACHE:
        _CACHE[key] = build_nc(reps, loop=loop, skip_dma=skip_dma,
                               hints=hints, pvswap=pvswap)
    in_maps = make_in_maps(np.asarray(x), np.asarray(w_qkv),
                           np.asarray(w_out), pvswap=pvswap)
    res = run_bass_kernel_spmd(_CACHE[key], in_maps,
                               core_ids=list(range(N_CORES)))
    parts = [np.asarray(res.results[c]["out"], np.float32)
             for c in range(N_CORES)]
    y = np.stack([parts[2 * n] + parts[2 * n + 1] for n in range(N)])
    y = y + np.asarray(b_out, np.float32)[None, :, None]
    return y.astype(np.float32), res


def kernel(x, w_qkv, w_out, b_out):
    y, _ = run(x, w_qkv, w_out, b_out)
    return y
